# revision 6
# baseline (speedup 1.0000x reference)
"""MinLSTM layer on 8 Trainium2 NeuronCores.

Math (equivalent to the log-space reference, done in linear space):
    f_pre = x @ W_f.T + b_f ; i_pre = x @ W_i.T + b_i ; h_pre = x @ W_h.T + b_h
    sf = sigmoid(f_pre) ; si = sigmoid(i_pre)
    f = sf / (sf + si)                       # normalized forget gate
    i = 1 - f                                # = si / (sf + si)
    g = max(sigmoid(h_pre), h_pre + 0.5)     # == exp(log_g), exactly
    h_t = f_t * h_{t-1} + i_t * g_t,  h_0 = 1
The gates satisfy f in (0,1), g > 0, so h stays in a tame range and the
recurrence is numerically stable in fp32.

Sharding: 8 cores = batch(4) x hidden-halves(2). Core c handles batch b=c//2,
hidden slice [(c%2)*512, (c%2+1)*512). No cross-core communication; the scan
runs along T inside each core via the DVE TensorTensorScan instruction
(state = f*state - mv per step, mv = (f-1)*g = -i*g).

Device layout: gates computed as [h_part, t_free] via out = W_sliceT.T @ xT;
host pre-transposes x and W (numpy, bf16) and re-transposes the [512, 4096]
per-core output back to [T, Dh]. Matmuls run in 512-wide t-chunks (one PSUM
bank); elementwise+scan run in up-to-1024-wide super-chunks.

Matmul operands are bf16 (max rel err vs the fp32 reference ~7e-3, within
the 2e-2 budget): halves the x/W DMA traffic vs fp32r and removes the
early-kernel DMA-starvation stalls.

Engine split: ACT does the three sigmoids off PSUM; the Pool engine (idle
otherwise) takes the g=max(sig,h+0.5) STT, the sf+si add, and the
mv=(f-1)*g STT; the DVE keeps reciprocal, f=sf*r, and the scan. This
shortens the post-last-matmul serial chain (the tail runs at half clock
once the PE goes idle - HAM drops the clock grant).

Scheduling notes:
- x and W live in per-k tiles (contraction slices) so the PE's dependency on
  each matmul is one 128KB DMA; at startup the PE chases the HBM stream.
- The first super-chunk is gate-major (f for all h-tiles, then i, then h),
  k-outer, in DMA priority order (x0[k], W_f[k]) -> x1 -> W_i -> W_h; later
  chunks are h-tile-major with per-(gate, half) PSUM accumulation.
- Warmup matmuls on a zeroed scratch tile fill the engine-preamble ->
  first-data window at full array duty so the PE's HAM clock gate reaches
  2.4 GHz as real matmuls start (N=128 warmups do NOT work: ~31% array duty
  never trips the activity monitor). The scratch memset runs on the Pool
  engine so warmups start as early as possible (HAM grants full clock
  ~5.1us after sustained PE activity begins).
- The final two t-chunks are 256 wide so the end-of-kernel serial chain
  (sigmoid -> g-max -> mv -> scan -> store) is short.
"""

import sys

for _p in ("/opt/trn_rl_repo",):
    if _p not in sys.path:
        sys.path.append(_p)

import numpy as np
import ml_dtypes

import concourse.bass as bass
import concourse.tile as tile
from concourse import bacc, mybir
from concourse.bass_utils import run_bass_kernel_spmd

B, T, DIN, DH = 4, 4096, 1024, 1024
N_CORES = 8
HSH = DH // 2          # 512 hidden channels per core
P = 128                # partitions
KT = DIN // P          # 8 contraction tiles
NT = 512               # matmul t-chunk (free dim, one PSUM bank)
IT = HSH // P          # 4 h-tiles per core
# elementwise/scan super-chunks (start, length); tail chunks smaller to
# shrink the end-of-kernel drain
CHUNKS = [(0, 1024), (1024, 1024), (2048, 1024), (3072, 512),
          (3584, 256), (3840, 256)]
N_WARM = 6             # warmup matmuls before the first real one

MM_DT = mybir.dt.bfloat16
NP_MM_DT = ml_dtypes.bfloat16

_COMPILED = None


def _build():
    AF = mybir.ActivationFunctionType
    OP = mybir.AluOpType
    f32 = mybir.dt.float32

    nc = bacc.Bacc("TRN2", target_bir_lowering=False, debug=False)

    xT = nc.dram_tensor("xT", [DIN, T], MM_DT, kind="ExternalInput").ap()
    wd = {g: nc.dram_tensor(f"w{g}", [DIN, HSH], MM_DT, kind="ExternalInput").ap()
          for g in ("f", "i", "h")}
    # packed per-partition scalars: [b_f | b_i | b_h | b_h+0.5], each (128, IT)
    biases = nc.dram_tensor("biases", [P, 4 * IT], f32, kind="ExternalInput").ap()
    out = nc.dram_tensor("out", [HSH, T], f32, kind="ExternalOutput").ap()

    # DRAM views: (KT*P, n) -> [p, k, n]
    xT_v = xT.rearrange("(k p) t -> p k t", p=P)
    w_v = {g: w.rearrange("(k p) h -> p k h", p=P) for g, w in wd.items()}

    with tile.TileContext(nc) as tc:
        with (
            tc.tile_pool(name="wpool", bufs=1) as wpool,
            tc.tile_pool(name="bpool", bufs=1) as bpool,
            tc.tile_pool(name="xpool", bufs=32) as xpool,
            tc.tile_pool(name="psum", bufs=8, space="PSUM") as pspool,
            tc.tile_pool(name="work", bufs=4) as work,
            tc.tile_pool(name="hpool", bufs=6) as hpool,
        ):
            bias_t = bpool.tile([P, 4 * IT], f32, tag="bias")

            # per-k weight tiles, resident all kernel
            wt = {g: [wpool.tile([P, HSH], MM_DT, tag=f"w{g}{k}", name=f"w{g}{k}_t")
                      for k in range(KT)] for g in ("f", "i", "h")}

            def dma_w(g):
                for k in range(KT):
                    nc.sync.dma_start(out=wt[g][k][:], in_=w_v[g][:, k, :])

            def x_ktiles(t0, nt):
                """One [P, nt] tile per contraction slice k of a t-chunk."""
                xs = []
                for k in range(KT):
                    xk = xpool.tile([P, nt], MM_DT, tag="xk", name="xk_t")
                    nc.sync.dma_start(out=xk[:], in_=xT_v[:, k, t0:t0 + nt])
                    xs.append(xk)
                return xs

            def bias_ap(kind, i):
                return bias_t[:, kind * IT + i:kind * IT + i + 1]

            def chain(i, sf, si, sg, gt, J, t0, ne):
                """Normalize gates, build i*g, scan, and store chunk.

                GPSIMD supports only software Add/Multiply (no STT/scan, no
                PSUM access), so it takes s=sf+si, i=si*r, w=i*g; the DVE
                keeps reciprocal, f=sf*r, and the scan state = f*state + w.
                """
                s = work.tile([P, ne], f32, tag="s", name="s_t")
                nc.gpsimd.tensor_add(s[:], sf[:, :ne], si[:, :ne])
                r = work.tile([P, ne], f32, tag="r", name="r_t")
                nc.vector.reciprocal_approx_fast(out=r[:], in_=s[:])
                nc.vector.tensor_mul(sf[:, :ne], sf[:, :ne], r[:])      # f
                nc.gpsimd.tensor_mul(si[:, :ne], si[:, :ne], r[:])      # i
                nc.gpsimd.tensor_mul(gt[:, :ne], si[:, :ne], gt[:, :ne])  # w=i*g
                hc = hpool.tile([P, ne], f32, tag="h", name=f"h{i}_t")
                init = 1.0 if J == 0 else hprev[i][:, -1:]
                nc.vector.tensor_tensor_scan(
                    hc[:], sf[:, :ne], gt[:, :ne], init,
                    op0=OP.mult, op1=OP.add)
                hprev[i] = hc
                nc.sync.dma_start(
                    out=out[i * P:(i + 1) * P, t0:t0 + ne], in_=hc[:])

            hprev = [None] * IT
            hsls = [slice(i * P, (i + 1) * P) for i in range(IT)]

            # Fill the preamble->first-data window with warmup matmuls on a
            # zeroed scratch tile so the PE's HAM clock gate ramps toward
            # 2.4 GHz before real matmuls start. Memset on Pool so the PE
            # can begin immediately after its preamble.
            scratch = bpool.tile([P, NT], MM_DT, tag="scratch")
            nc.gpsimd.memset(scratch[:].bitcast(mybir.dt.uint16), 0)
            pswarm = pspool.tile([P, NT], f32, tag="ps", name="pswarm_t")
            for _ in range(N_WARM):
                nc.tensor.matmul(pswarm[:], lhsT=scratch[:, :P], rhs=scratch[:],
                                 start=True, stop=True)

            # ---- J0: gate-major, k-outer; PE chases the input DMA stream ----
            t0, ne = CHUNKS[0]
            nhalf = ne // NT
            # priority order: (x_h0[k], W_f[k]) pairs, x_h1, W_i, W_h
            xcs = [[xpool.tile([P, NT], MM_DT, tag="xk", name="xk_t")
                    for _ in range(KT)] for _ in range(nhalf)]
            for k in range(KT):
                nc.sync.dma_start(out=xcs[0][k][:], in_=xT_v[:, k, t0:t0 + NT])
                nc.sync.dma_start(out=wt["f"][k][:], in_=w_v["f"][:, k, :])
                if k == 0:
                    # bias is tiny and first needed by the ACTs; issue it
                    # after the first matmul's dependencies
                    nc.sync.dma_start(out=bias_t[:], in_=biases[:])
            for h in range(1, nhalf):
                th = t0 + h * NT
                for k in range(KT):
                    nc.sync.dma_start(out=xcs[h][k][:], in_=xT_v[:, k, th:th + NT])
            dma_w("i")
            dma_w("h")

            sf = [work.tile([P, ne], f32, tag="sf", name="sf_t") for _ in range(IT)]
            si = [work.tile([P, ne], f32, tag="si", name="si_t") for _ in range(IT)]
            sg = [work.tile([P, ne], f32, tag="sg", name="sg_t") for _ in range(IT)]
            gt = [work.tile([P, ne], f32, tag="gt", name="gt_t") for _ in range(IT)]
            for gate, dsts, bk in (("f", sf, 0), ("i", si, 1), ("h", sg, 2)):
                for half in range(nhalf):
                    esl = slice(half * NT, (half + 1) * NT)
                    psts = [pspool.tile([P, NT], f32, tag="ps", name="ps_t")
                            for _ in range(IT)]
                    for k in range(KT):
                        for pst, hsl in zip(psts, hsls):
                            nc.tensor.matmul(
                                pst[:], lhsT=wt[gate][k][:, hsl],
                                rhs=xcs[half][k][:],
                                start=(k == 0), stop=(k == KT - 1))
                    for i in range(IT):
                        nc.scalar.activation(dsts[i][:, esl], psts[i][:], AF.Sigmoid,
                                             bias=bias_ap(bk, i), scale=1.0)
                        if gate == "h":
                            # DVE: GPSIMD cannot read PSUM
                            nc.vector.scalar_tensor_tensor(
                                gt[i][:, esl], psts[i][:], bias_ap(3, i),
                                sg[i][:, esl], op0=OP.add, op1=OP.max)
            for i in range(IT):
                chain(i, sf[i], si[i], sg[i], gt[i], 0, t0, ne)

            # ---- J1+: h-tile-major units ----
            for J, (t0, ne) in enumerate(CHUNKS[1:], start=1):
                nfull, rem = divmod(ne, NT)
                widths = [NT] * nfull + ([rem] if rem else [])
                xcs = []
                toff = t0
                for w_ in widths:
                    xcs.append((x_ktiles(toff, w_), toff - t0, w_))
                    toff += w_
                for i in range(IT):
                    hsl = hsls[i]
                    sf = work.tile([P, ne], f32, tag="sf", name="sf_t")
                    si = work.tile([P, ne], f32, tag="si", name="si_t")
                    sg = work.tile([P, ne], f32, tag="sg", name="sg_t")
                    gt = work.tile([P, ne], f32, tag="gt", name="gt_t")
                    for xks, eoff, w_ in xcs:
                        esl = slice(eoff, eoff + w_)
                        for gate, dst, bk in (("f", sf, 0), ("i", si, 1),
                                              ("h", sg, 2)):
                            # full-width tile: PSUM zeroing on start=True is
                            # 2KB-bank granular, so sub-bank tiles must not
                            # share a bank with a live accumulation
                            pst = pspool.tile([P, NT], f32, tag="ps", name="ps_t")
                            for k in range(KT):
                                nc.tensor.matmul(
                                    pst[:, :w_], lhsT=wt[gate][k][:, hsl],
                                    rhs=xks[k][:],
                                    start=(k == 0), stop=(k == KT - 1))
                            nc.scalar.activation(dst[:, esl], pst[:, :w_],
                                                 AF.Sigmoid, bias=bias_ap(bk, i),
                                                 scale=1.0)
                            if gate == "h":
                                # DVE: GPSIMD cannot read PSUM
                                nc.vector.scalar_tensor_tensor(
                                    gt[:, esl], pst[:, :w_], bias_ap(3, i),
                                    sg[:, esl], op0=OP.add, op1=OP.max)
                    chain(i, sf, si, sg, gt, J, t0, ne)

    nc.compile()
    return nc


def _in_maps(x, W_f, b_f, W_i, b_i, W_h, b_h):
    x = np.asarray(x, np.float32)
    wT = {g: np.ascontiguousarray(np.asarray(w, np.float32).T).astype(NP_MM_DT)
          for g, w in (("f", W_f), ("i", W_i), ("h", W_h))}
    bs = {g: np.asarray(b, np.float32) for g, b in (("f", b_f), ("i", b_i), ("h", b_h))}

    maps = []
    for c in range(N_CORES):
        b, hh = divmod(c, 2)
        hsl = slice(hh * HSH, (hh + 1) * HSH)
        bias_pack = np.concatenate([
            bs["f"][hsl].reshape(IT, P).T,
            bs["i"][hsl].reshape(IT, P).T,
            bs["h"][hsl].reshape(IT, P).T,
            (bs["h"][hsl] + 0.5).reshape(IT, P).T,
        ], axis=1)
        maps.append({
            "xT": np.ascontiguousarray(x[b].T).astype(NP_MM_DT),
            "wf": np.ascontiguousarray(wT["f"][:, hsl]),
            "wi": np.ascontiguousarray(wT["i"][:, hsl]),
            "wh": np.ascontiguousarray(wT["h"][:, hsl]),
            "biases": np.ascontiguousarray(bias_pack, dtype=np.float32),
        })
    return maps


def kernel(x, W_f, b_f, W_i, b_i, W_h, b_h):
    global _COMPILED
    if _COMPILED is None:
        _COMPILED = _build()
    nc = _COMPILED

    res = run_bass_kernel_spmd(
        nc, _in_maps(x, W_f, b_f, W_i, b_i, W_h, b_h), list(range(N_CORES)))

    full = np.empty((B, T, DH), np.float32)
    for c in range(N_CORES):
        b, hh = divmod(c, 2)
        full[b, :, hh * HSH:(hh + 1) * HSH] = res.results[c]["out"].T
    return full


# revision 8
# speedup vs baseline: 1.2031x; 1.2031x over previous
"""MinLSTM layer on 8 Trainium2 NeuronCores.

Math (equivalent to the log-space reference, done in linear space):
    f_pre = x @ W_f.T + b_f ; i_pre = x @ W_i.T + b_i ; h_pre = x @ W_h.T + b_h
    sf = sigmoid(f_pre) ; si = sigmoid(i_pre)
    f = sf / (sf + si)                       # normalized forget gate
    i = 1 - f                                # = si / (sf + si)
    g = max(sigmoid(h_pre), h_pre + 0.5)     # == exp(log_g), exactly
    h_t = f_t * h_{t-1} + i_t * g_t,  h_0 = 1
The gates satisfy f in (0,1), g > 0, so h stays in a tame range and the
recurrence is numerically stable in fp32.

Sharding: 8 cores = batch(4) x hidden-halves(2). Core c handles batch b=c//2,
hidden slice [(c%2)*512, (c%2+1)*512). No cross-core communication; the scan
runs along T inside each core via the DVE TensorTensorScan instruction
(state = f*state - mv per step, mv = (f-1)*g = -i*g).

Device layout: gates computed as [h_part, t_free] via out = W_sliceT.T @ xT;
host pre-transposes x and W (numpy, bf16) and re-transposes the [512, 4096]
per-core output back to [T, Dh]. Matmuls run in 512-wide t-chunks (one PSUM
bank); elementwise+scan run in up-to-1024-wide super-chunks.

Matmul operands are bf16 (max rel err vs the fp32 reference ~7e-3, within
the 2e-2 budget): halves the x/W DMA traffic vs fp32r and removes the
early-kernel DMA-starvation stalls.

Engine split: ACT does the three sigmoids off PSUM; the Pool engine (idle
otherwise) takes the g=max(sig,h+0.5) STT, the sf+si add, and the
mv=(f-1)*g STT; the DVE keeps reciprocal, f=sf*r, and the scan. This
shortens the post-last-matmul serial chain (the tail runs at half clock
once the PE goes idle - HAM drops the clock grant).

Scheduling notes:
- x and W live in per-k tiles (contraction slices) so the PE's dependency on
  each matmul is one 128KB DMA; at startup the PE chases the HBM stream.
- The first super-chunk is gate-major (f for all h-tiles, then i, then h),
  k-outer, in DMA priority order (x0[k], W_f[k]) -> x1 -> W_i -> W_h; later
  chunks are h-tile-major with per-(gate, half) PSUM accumulation.
- Warmup matmuls on a zeroed scratch tile fill the engine-preamble ->
  first-data window at full array duty so the PE's HAM clock gate reaches
  2.4 GHz as real matmuls start (N=128 warmups do NOT work: ~31% array duty
  never trips the activity monitor). The scratch memset runs on the Pool
  engine so warmups start as early as possible (HAM grants full clock
  ~5.1us after sustained PE activity begins).
- The final two t-chunks are 256 wide so the end-of-kernel serial chain
  (sigmoid -> g-max -> mv -> scan -> store) is short.
"""

import sys

for _p in ("/opt/trn_rl_repo",):
    if _p not in sys.path:
        sys.path.append(_p)

import numpy as np
import ml_dtypes

import concourse.bass as bass
import concourse.tile as tile
from concourse import bacc, mybir
from concourse.bass_utils import run_bass_kernel_spmd

B, T, DIN, DH = 4, 4096, 1024, 1024
N_CORES = 8
HSH = DH // 2          # 512 hidden channels per core
P = 128                # partitions
KT = DIN // P          # 8 contraction tiles
NT = 512               # matmul t-chunk (free dim, one PSUM bank)
IT = HSH // P          # 4 h-tiles per core
# elementwise/scan super-chunks (start, length); tail chunks smaller to
# shrink the end-of-kernel drain
CHUNKS = [(0, 1024), (1024, 1024), (2048, 1024), (3072, 512),
          (3584, 256), (3840, 256)]
N_WARM = 6             # warmup matmuls before the first real one

MM_DT = mybir.dt.bfloat16
NP_MM_DT = ml_dtypes.bfloat16

_COMPILED = None


def _build():
    AF = mybir.ActivationFunctionType
    OP = mybir.AluOpType
    f32 = mybir.dt.float32

    nc = bacc.Bacc("TRN2", target_bir_lowering=False, debug=False)

    xT = nc.dram_tensor("xT", [DIN, T], MM_DT, kind="ExternalInput").ap()
    wd = {g: nc.dram_tensor(f"w{g}", [DIN, HSH], MM_DT, kind="ExternalInput").ap()
          for g in ("f", "i", "h")}
    # packed per-partition scalars: [b_f | b_i | b_h | b_h+0.5], each (128, IT)
    biases = nc.dram_tensor("biases", [P, 4 * IT], f32, kind="ExternalInput").ap()
    out = nc.dram_tensor("out", [HSH, T], f32, kind="ExternalOutput").ap()

    # DRAM views: (KT*P, n) -> [p, k, n]
    xT_v = xT.rearrange("(k p) t -> p k t", p=P)
    w_v = {g: w.rearrange("(k p) h -> p k h", p=P) for g, w in wd.items()}

    with tile.TileContext(nc) as tc:
        with (
            tc.tile_pool(name="wpool", bufs=1) as wpool,
            tc.tile_pool(name="bpool", bufs=1) as bpool,
            tc.tile_pool(name="xpool", bufs=32) as xpool,
            tc.tile_pool(name="psum", bufs=8, space="PSUM") as pspool,
            tc.tile_pool(name="work", bufs=4) as work,
            tc.tile_pool(name="hpool", bufs=6) as hpool,
        ):
            bias_t = bpool.tile([P, 4 * IT], f32, tag="bias")

            # per-k weight tiles, resident all kernel
            wt = {g: [wpool.tile([P, HSH], MM_DT, tag=f"w{g}{k}", name=f"w{g}{k}_t")
                      for k in range(KT)] for g in ("f", "i", "h")}

            def dma_w(g):
                for k in range(KT):
                    nc.sync.dma_start(out=wt[g][k][:], in_=w_v[g][:, k, :])

            def x_ktiles(t0, nt):
                """One [P, nt] tile per contraction slice k of a t-chunk."""
                xs = []
                for k in range(KT):
                    xk = xpool.tile([P, nt], MM_DT, tag="xk", name="xk_t")
                    nc.sync.dma_start(out=xk[:], in_=xT_v[:, k, t0:t0 + nt])
                    xs.append(xk)
                return xs

            def bias_ap(kind, i):
                return bias_t[:, kind * IT + i:kind * IT + i + 1]

            def chain(i, sf, si, sg, gt, J, t0, ne):
                """Normalize gates, build -i*g, scan, and store chunk.

                GPSIMD software ops are launch-dominated (~2-3ns/elem), so
                it only takes the dependency-head add (s=sf+si) on big
                chunks, off the DVE; the serial tail chunks keep everything
                on the DVE.
                """
                s = work.tile([P, ne], f32, tag="s", name="s_t")
                eng = nc.gpsimd if ne >= 1024 else nc.vector
                eng.tensor_add(s[:], sf[:, :ne], si[:, :ne])
                r = work.tile([P, ne], f32, tag="r", name="r_t")
                nc.vector.reciprocal_approx_fast(out=r[:], in_=s[:])
                nc.vector.tensor_mul(sf[:, :ne], sf[:, :ne], r[:])      # f
                nc.vector.scalar_tensor_tensor(                # mv=(f-1)*g
                    gt[:, :ne], sf[:, :ne], 1.0, gt[:, :ne],
                    op0=OP.subtract, op1=OP.mult)
                hc = hpool.tile([P, ne], f32, tag="h", name=f"h{i}_t")
                init = 1.0 if J == 0 else hprev[i][:, -1:]
                nc.vector.tensor_tensor_scan(
                    hc[:], sf[:, :ne], gt[:, :ne], init,
                    op0=OP.mult, op1=OP.subtract)
                hprev[i] = hc
                nc.sync.dma_start(
                    out=out[i * P:(i + 1) * P, t0:t0 + ne], in_=hc[:])

            hprev = [None] * IT
            hsls = [slice(i * P, (i + 1) * P) for i in range(IT)]

            # Fill the preamble->first-data window with warmup matmuls on a
            # zeroed scratch tile so the PE's HAM clock gate ramps toward
            # 2.4 GHz before real matmuls start. Memset on Pool so the PE
            # can begin immediately after its preamble.
            scratch = bpool.tile([P, NT], MM_DT, tag="scratch")
            nc.scalar.memzero(scratch[:])
            pswarm = pspool.tile([P, NT], f32, tag="ps", name="pswarm_t")
            for _ in range(N_WARM):
                nc.tensor.matmul(pswarm[:], lhsT=scratch[:, :P], rhs=scratch[:],
                                 start=True, stop=True)

            # ---- J0: gate-major, k-outer; PE chases the input DMA stream ----
            t0, ne = CHUNKS[0]
            nhalf = ne // NT
            # priority order: (x_h0[k], W_f[k]) pairs, x_h1, W_i, W_h
            xcs = [[xpool.tile([P, NT], MM_DT, tag="xk", name="xk_t")
                    for _ in range(KT)] for _ in range(nhalf)]
            for k in range(KT):
                nc.sync.dma_start(out=xcs[0][k][:], in_=xT_v[:, k, t0:t0 + NT])
                nc.sync.dma_start(out=wt["f"][k][:], in_=w_v["f"][:, k, :])
                if k == 0:
                    # bias is tiny and first needed by the ACTs; issue it
                    # after the first matmul's dependencies
                    nc.sync.dma_start(out=bias_t[:], in_=biases[:])
            for h in range(1, nhalf):
                th = t0 + h * NT
                for k in range(KT):
                    nc.sync.dma_start(out=xcs[h][k][:], in_=xT_v[:, k, th:th + NT])
            dma_w("i")
            dma_w("h")

            sf = [work.tile([P, ne], f32, tag="sf", name="sf_t") for _ in range(IT)]
            si = [work.tile([P, ne], f32, tag="si", name="si_t") for _ in range(IT)]
            sg = [work.tile([P, ne], f32, tag="sg", name="sg_t") for _ in range(IT)]
            gt = [work.tile([P, ne], f32, tag="gt", name="gt_t") for _ in range(IT)]
            for gate, dsts, bk in (("f", sf, 0), ("i", si, 1), ("h", sg, 2)):
                for half in range(nhalf):
                    esl = slice(half * NT, (half + 1) * NT)
                    psts = [pspool.tile([P, NT], f32, tag="ps", name="ps_t")
                            for _ in range(IT)]
                    for k in range(KT):
                        for pst, hsl in zip(psts, hsls):
                            nc.tensor.matmul(
                                pst[:], lhsT=wt[gate][k][:, hsl],
                                rhs=xcs[half][k][:],
                                start=(k == 0), stop=(k == KT - 1))
                    for i in range(IT):
                        nc.scalar.activation(dsts[i][:, esl], psts[i][:], AF.Sigmoid,
                                             bias=bias_ap(bk, i), scale=1.0)
                        if gate == "h":
                            # DVE: GPSIMD cannot read PSUM
                            nc.vector.scalar_tensor_tensor(
                                gt[i][:, esl], psts[i][:], bias_ap(3, i),
                                sg[i][:, esl], op0=OP.add, op1=OP.max)
            for i in range(IT):
                chain(i, sf[i], si[i], sg[i], gt[i], 0, t0, ne)

            # ---- J1+: h-tile-major units ----
            for J, (t0, ne) in enumerate(CHUNKS[1:], start=1):
                nfull, rem = divmod(ne, NT)
                widths = [NT] * nfull + ([rem] if rem else [])
                xcs = []
                toff = t0
                for w_ in widths:
                    xcs.append((x_ktiles(toff, w_), toff - t0, w_))
                    toff += w_
                for i in range(IT):
                    hsl = hsls[i]
                    sf = work.tile([P, ne], f32, tag="sf", name="sf_t")
                    si = work.tile([P, ne], f32, tag="si", name="si_t")
                    sg = work.tile([P, ne], f32, tag="sg", name="sg_t")
                    gt = work.tile([P, ne], f32, tag="gt", name="gt_t")
                    for xks, eoff, w_ in xcs:
                        esl = slice(eoff, eoff + w_)
                        for gate, dst, bk in (("f", sf, 0), ("i", si, 1),
                                              ("h", sg, 2)):
                            # full-width tile: PSUM zeroing on start=True is
                            # 2KB-bank granular, so sub-bank tiles must not
                            # share a bank with a live accumulation
                            pst = pspool.tile([P, NT], f32, tag="ps", name="ps_t")
                            for k in range(KT):
                                nc.tensor.matmul(
                                    pst[:, :w_], lhsT=wt[gate][k][:, hsl],
                                    rhs=xks[k][:],
                                    start=(k == 0), stop=(k == KT - 1))
                            nc.scalar.activation(dst[:, esl], pst[:, :w_],
                                                 AF.Sigmoid, bias=bias_ap(bk, i),
                                                 scale=1.0)
                            if gate == "h":
                                # DVE: GPSIMD cannot read PSUM
                                nc.vector.scalar_tensor_tensor(
                                    gt[:, esl], pst[:, :w_], bias_ap(3, i),
                                    sg[:, esl], op0=OP.add, op1=OP.max)
                    chain(i, sf, si, sg, gt, J, t0, ne)

    nc.compile()
    return nc


def _in_maps(x, W_f, b_f, W_i, b_i, W_h, b_h):
    x = np.asarray(x, np.float32)
    wT = {g: np.ascontiguousarray(np.asarray(w, np.float32).T).astype(NP_MM_DT)
          for g, w in (("f", W_f), ("i", W_i), ("h", W_h))}
    bs = {g: np.asarray(b, np.float32) for g, b in (("f", b_f), ("i", b_i), ("h", b_h))}

    maps = []
    for c in range(N_CORES):
        b, hh = divmod(c, 2)
        hsl = slice(hh * HSH, (hh + 1) * HSH)
        bias_pack = np.concatenate([
            bs["f"][hsl].reshape(IT, P).T,
            bs["i"][hsl].reshape(IT, P).T,
            bs["h"][hsl].reshape(IT, P).T,
            (bs["h"][hsl] + 0.5).reshape(IT, P).T,
        ], axis=1)
        maps.append({
            "xT": np.ascontiguousarray(x[b].T).astype(NP_MM_DT),
            "wf": np.ascontiguousarray(wT["f"][:, hsl]),
            "wi": np.ascontiguousarray(wT["i"][:, hsl]),
            "wh": np.ascontiguousarray(wT["h"][:, hsl]),
            "biases": np.ascontiguousarray(bias_pack, dtype=np.float32),
        })
    return maps


def kernel(x, W_f, b_f, W_i, b_i, W_h, b_h):
    global _COMPILED
    if _COMPILED is None:
        _COMPILED = _build()
    nc = _COMPILED

    res = run_bass_kernel_spmd(
        nc, _in_maps(x, W_f, b_f, W_i, b_i, W_h, b_h), list(range(N_CORES)))

    full = np.empty((B, T, DH), np.float32)
    for c in range(N_CORES):
        b, hh = divmod(c, 2)
        full[b, :, hh * HSH:(hh + 1) * HSH] = res.results[c]["out"].T
    return full
